# revision 10
# baseline (speedup 1.0000x reference)
"""Top-k gated mixture of linear maps (MoE routing) on 8 TRN2 NeuronCores.

Expert-parallel sharding: the 16 charts are assigned 2-per-core. Routing
(top-2 chart selection + gate normalization, an 8192x16 argmax — 0.003% of
the FLOPs) runs on host as part of the dispatch/sharding step; each core
receives the tokens routed to its two charts as pre-transposed column
blocks plus per-token gate scales, with its two charts' W^T resident in
SBUF. The device performs all matmul FLOPs (float32r, full PE rate) and
the gate scaling; the host unshard step sums each token's two chart
contributions.

self-contained: only imports concourse (globally installed) + numpy.
"""

import numpy as np

P = 128
D = 1024  # DIM_Q == DIM_U
NUM_CHARTS = 16
TOP_K = 2
N_CORES = 8
UH = 512  # moving-operand free-dim per matmul (fp32 max)

_NC_CACHE: dict = {}


def _build(cap: int):
    """Build + compile the SPMD single-core program (same on all 8 cores).

    Inputs per core:
      xt [8, 128, 2*cap] f32r : X^T for the core's 2 chart slots; chunk qc,
                                partition p holds q-dim index qc*128+p;
                                columns s*cap..s*cap+n_s are slot s's tokens.
      wt [2, 8, 128, 1024] f32r : W^T of the 2 charts, q-chunked the same way.
      sc [128, 2*(cap/128)] f32 : gate scale of token (slot s, tile tt,
                                  partition p) at column s*NT+tt.
    Output:
      y [2*cap, 1024] f32 : scaled chart outputs, row = slot*cap + token idx.
    """
    import concourse.mybir as mybir
    from concourse import bacc
    from concourse.tile import TileContext

    NT = cap // P
    f32 = mybir.dt.float32
    f32r = mybir.dt.float32r

    nc = bacc.Bacc("TRN2", target_bir_lowering=False, debug=False)
    xt = nc.dram_tensor("xt", [8, P, 2 * cap], f32r, kind="ExternalInput")
    wt = nc.dram_tensor("wt", [2, 8, P, D], f32r, kind="ExternalInput")
    sc = nc.dram_tensor("sc", [P, 2 * NT], f32, kind="ExternalInput")
    y = nc.dram_tensor("y", [2 * cap, D], f32, kind="ExternalOutput")

    with TileContext(nc) as tc:
        with (
            tc.tile_pool(name="wpool", bufs=1) as wpool,
            tc.tile_pool(name="xpool", bufs=6) as xpool,
            tc.tile_pool(name="spool", bufs=1) as spool,
            tc.tile_pool(name="opool", bufs=6) as opool,
            tc.tile_pool(name="psum", bufs=6, space="PSUM") as psum,
        ):
            sc_sb = spool.tile([P, 2 * NT], f32)
            nc.sync.dma_start(sc_sb[:], sc[:])

            # W^T resident in SBUF: per slot 4 tiles of 2 q-chunks each.
            # Separate tiles give the scheduler per-chunk dependencies, so
            # matmul qc can start as soon as its own chunk has landed; slot
            # 1's chunks are issued interleaved with slot-0 X loads below.
            w_sb = {s: [None] * 4 for s in range(2)}

            def _load_w(s, h):
                t = wpool.tile([P, 2 * D], f32r, tag=f"w_{s}_{h}", name=f"w_{s}_{h}")
                nc.sync.dma_start(
                    t[:].rearrange("p (qc u) -> p qc u", qc=2),
                    wt[s, 2 * h : 2 * h + 2].rearrange("qc p u -> p qc u"),
                )
                w_sb[s][h] = t

            for h in range(4):
                _load_w(0, h)

            for s in range(2):
                for tt in range(NT):
                    col = s * cap + tt * P
                    # one DMA per token tile: [8, 128, 128] -> [128, 8*128]
                    xtile = xpool.tile([P, 8 * P], f32r, tag="x", name=f"x_{s}_{tt}")
                    nc.sync.dma_start(
                        xtile[:].rearrange("p (qc t) -> p qc t", qc=8),
                        xt[:, :, col : col + P].rearrange("qc p t -> p qc t"),
                    )
                    if s == 0 and 1 <= tt <= 4:
                        # trickle slot-1 W in behind the early slot-0 X loads
                        _load_w(1, tt - 1)
                    if s == 0 and tt == NT - 1:
                        for h in range(4):
                            if w_sb[1][h] is None:
                                _load_w(1, h)
                    pss = [
                        psum.tile([P, UH], f32, tag="ps", name=f"ps_{s}_{tt}_{u}")
                        for u in range(2)
                    ]
                    # qc outer / u-half inner: consecutive matmuls share the
                    # stationary operand, accumulation interleaves across 2
                    # PSUM banks.
                    for qc in range(8):
                        for uh in range(2):
                            nc.tensor.matmul(
                                pss[uh][:],
                                lhsT=xtile[:, qc * P : (qc + 1) * P],
                                rhs=w_sb[s][qc // 2][
                                    :,
                                    (qc % 2) * D + uh * UH : (qc % 2) * D
                                    + (uh + 1) * UH,
                                ],
                                start=(qc == 0),
                                stop=(qc == 7),
                            )
                    ot = opool.tile([P, D], f32, tag="o", name=f"o_{s}_{tt}")
                    for uh in range(2):
                        nc.vector.tensor_scalar_mul(
                            ot[:, uh * UH : (uh + 1) * UH],
                            pss[uh][:],
                            sc_sb[:, s * NT + tt : s * NT + tt + 1],
                        )
                    # Y stores issue from the (otherwise idle) ACT queue so
                    # their waits don't head-of-line-block the X loads on SP.
                    nc.scalar.dma_start(y[col : col + P, :], ot[:])
    nc.compile()
    return nc


def _get_nc(cap: int):
    if cap not in _NC_CACHE:
        _NC_CACHE[cap] = _build(cap)
    return _NC_CACHE[cap]


def _route(weights: np.ndarray):
    """Host router: top-2 charts + normalized gates (matches jax.lax.top_k
    tie-breaking: lower index first)."""
    b = weights.shape[0]
    ar = np.arange(b)
    i1 = np.argmax(weights, axis=1)
    v1 = weights[ar, i1]
    w2 = weights.copy()
    w2[ar, i1] = -np.inf
    i2 = np.argmax(w2, axis=1)
    v2 = w2[ar, i2]
    s = np.clip(v1 + v2, 1e-8, None)
    return i1, i2, v1 / s, v2 / s


def kernel(q, weights, W_stack):
    q = np.ascontiguousarray(np.asarray(q, dtype=np.float32))
    weights = np.ascontiguousarray(np.asarray(weights, dtype=np.float32))
    W_stack = np.asarray(W_stack, dtype=np.float32)

    B = q.shape[0]
    i1, i2, g1, g2 = _route(weights)

    # flatten the (token, chart) pairs and group by chart
    flat_chart = np.concatenate([i1, i2])
    flat_tok = np.concatenate([np.arange(B), np.arange(B)])
    flat_gate = np.concatenate([g1, g2]).astype(np.float32)
    order = np.argsort(flat_chart, kind="stable")
    counts = np.bincount(flat_chart, minlength=NUM_CHARTS)
    starts = np.zeros(NUM_CHARTS + 1, dtype=np.int64)
    starts[1:] = np.cumsum(counts)

    cap = max(1152, int(-(-counts.max() // P)) * P)
    NT = cap // P

    Wt = np.ascontiguousarray(W_stack.transpose(0, 2, 1))  # [C, q, u]

    in_maps = []
    pos = np.empty(2 * B, dtype=np.int64)  # pair j -> global Y row
    for core in range(N_CORES):
        xt = np.zeros((8, P, 2 * cap), dtype=np.float32)
        scm = np.zeros((P, 2 * NT), dtype=np.float32)
        for slot in range(2):
            c = core * 2 + slot
            sel = order[starts[c] : starts[c + 1]]
            toks = flat_tok[sel]
            n = len(sel)
            xt[:, :, slot * cap : slot * cap + n] = q[toks].T.reshape(8, P, n)
            sc_flat = np.zeros(cap, dtype=np.float32)
            sc_flat[:n] = flat_gate[sel]
            scm[:, slot * NT : (slot + 1) * NT] = sc_flat.reshape(NT, P).T
            pos[sel] = core * (2 * cap) + slot * cap + np.arange(n)
        in_maps.append(
            {
                "xt": xt,
                "wt": Wt[core * 2 : core * 2 + 2].reshape(2, 8, P, D),
                "sc": scm,
            }
        )

    from concourse.bass_utils import run_bass_kernel_spmd

    nc = _get_nc(cap)
    res = run_bass_kernel_spmd(nc, in_maps, core_ids=list(range(N_CORES)))
    Yg = np.concatenate([np.asarray(res.results[i]["y"]) for i in range(N_CORES)], axis=0)

    out = Yg[pos[:B]] + Yg[pos[B:]]
    return np.ascontiguousarray(out, dtype=np.float32)


# revision 22
# speedup vs baseline: 1.2893x; 1.2893x over previous
"""Top-k gated mixture of linear maps (MoE routing) on 8 TRN2 NeuronCores.

Expert-parallel sharding: the 16 charts are assigned 2-per-core (balanced
pairing: largest chart with smallest). Routing (top-2 chart selection +
gate normalization, an 8192x16 argmax — 0.003% of the FLOPs) runs on host
as part of the dispatch/sharding step; each core receives the tokens
routed to its two charts as transposed column blocks plus per-token gate
scales, with its two charts' residual weights resident in SBUF.

Residual decomposition: with gates g1+g2 = s, algebraically
    out = g1*W_c1 q + g2*W_c2 q = g1*(W_c1+I) q + g2*(W_c2+I) q - s*q.
The device computes the (W+I) matmuls in bf16 — since W ~= -I + 0.01*N,
the residual W+I is ~100x smaller than q's projection, so bf16 rounding
error scales with the residual, not the output (measured ~6e-4 scale-
relative absmax vs fp32 reference). Accumulation is fp32 in PSUM, gate
scaling on-device in fp32, and the exact -s*q correction is applied in
fp32 during the host unshard/combine step.

Matmul orientation: residual W^T chunks are the stationary operand and
token columns are the moving operand (out = Y^T tiles [u, tokens]), so
token capacities need no 128-alignment — per-slot capacity is exactly
the max chart load and the only padded work is capacity skew.

self-contained: only imports concourse (globally installed) + numpy.
"""

import numpy as np

P = 128
D = 1024  # DIM_Q == DIM_U
NUM_CHARTS = 16
TOP_K = 2
N_CORES = 8
MAX_MM = 512  # moving-operand / PSUM-bank limit per matmul

_NC_CACHE: dict = {}


def _blocks(cap0: int, cap1: int):
    """Static token-block list: (slot, col0, length) covering [0, cap0+cap1),
    slot-contiguous, each block <= MAX_MM."""
    out = []
    col = 0
    for s, cap in ((0, cap0), (1, cap1)):
        left = cap
        while left > 0:
            ln = min(MAX_MM, left)
            out.append((s, col, ln))
            col += ln
            left -= ln
    return out


def _build(cap0: int, cap1: int):
    """Build + compile the SPMD single-core program (same on all 8 cores).

    Inputs per core (CT = cap0 + cap1):
      xt  [8, 128, CT] bf16 : X^T q-chunked: xt[qc, p, col] =
                              q[token(col)][qc*128+p]. Slot 0 tokens in
                              columns [0, cap0), slot 1 after (zero-pad).
      wt  [2, 8, 128, 1024] bf16 : (W+I)^T of the 2 charts, q-chunked the
                              same way: wt[s, qc, p, u].
      scv [1, CT] f32 : gate scale of token(col); broadcast across
                              partitions on device via k=1 matmuls.
    Output:
      y [1024, CT] f32 : Y^T — gate-scaled residual chart outputs.
    """
    import concourse.mybir as mybir
    from concourse import bacc
    from concourse.tile import TileContext

    CT = cap0 + cap1
    f32 = mybir.dt.float32
    bf16 = mybir.dt.bfloat16
    blocks = _blocks(cap0, cap1)

    nc = bacc.Bacc("TRN2", target_bir_lowering=False, debug=False)
    xt = nc.dram_tensor("xt", [8, P, CT], bf16, kind="ExternalInput")
    wt = nc.dram_tensor("wt", [2, 8, P, D], bf16, kind="ExternalInput")
    scv = nc.dram_tensor("scv", [1, CT], f32, kind="ExternalInput")
    y = nc.dram_tensor("y", [D, CT], f32, kind="ExternalOutput")

    with TileContext(nc) as tc:
        with (
            tc.tile_pool(name="wpool", bufs=1) as wpool,
            tc.tile_pool(name="xpool", bufs=1) as xpool,
            tc.tile_pool(name="spool", bufs=1) as spool,
            tc.tile_pool(name="opool", bufs=2) as opool,
            tc.tile_pool(name="psum", bufs=8, space="PSUM") as psum,
        ):
            # gate-scale broadcast [1, CT] -> [128, CT] via k=1 matmuls with
            # a ones column; cheap, and warms the PE during the preload.
            sc_row = spool.tile([1, CT], f32, name="sc_row")
            nc.scalar.dma_start(sc_row[:], scv[:])
            ones = spool.tile([1, P], f32, name="ones")
            nc.vector.memset(ones[:], 1.0)
            sc_sb = spool.tile([P, CT], f32, name="sc_sb")
            for bi, (_s, c0, ln) in enumerate(blocks):
                bps = psum.tile([P, ln], f32, tag="ps", name=f"bps_{bi}")
                nc.tensor.matmul(
                    bps[:],
                    lhsT=ones[:],
                    rhs=sc_row[:, c0 : c0 + ln],
                    start=True,
                    stop=True,
                )
                nc.vector.tensor_copy(sc_sb[:, c0 : c0 + ln], bps[:])

            # X^T resident: 8 q-chunk tiles [128, CT]; residual W^T resident:
            # 8 per-q-chunk tiles per slot (fine-grained dependencies). Issue
            # order: (w0,x,w1) per q-chunk so accumulation over qc can chase
            # the arrival front.
            x_sb = [None] * 8
            w_sb = {0: [None] * 8, 1: [None] * 8}

            def _load_x(qc):
                t = xpool.tile([P, CT], bf16, tag=f"x_{qc}", name=f"x_{qc}")
                nc.sync.dma_start(t[:], xt[qc])
                x_sb[qc] = t

            def _load_w(s, qc):
                t = wpool.tile([P, D], bf16, tag=f"w_{s}_{qc}", name=f"w_{s}_{qc}")
                nc.sync.dma_start(t[:], wt[s, qc])
                w_sb[s][qc] = t

            for qc in range(8):
                _load_w(0, qc)
                _load_x(qc)
                _load_w(1, qc)

            for u in range(8):
                pss = [
                    psum.tile([P, ln], f32, tag="ps", name=f"ps_{u}_{bi}")
                    for bi, (_s, _c, ln) in enumerate(blocks)
                ]
                # qc outer, blocks inner: the stationary operand changes only
                # twice per qc (once per slot); each load streams ~cap tokens.
                for qc in range(8):
                    for bi, (s, c0, ln) in enumerate(blocks):
                        nc.tensor.matmul(
                            pss[bi][:],
                            lhsT=w_sb[s][qc][:, u * P : (u + 1) * P],
                            rhs=x_sb[qc][:, c0 : c0 + ln],
                            start=(qc == 0),
                            stop=(qc == 7),
                        )
                # per-block eviction + store: the Y^T DMA for a block starts
                # as soon as its psum is scaled (ACT queue, so the waits don't
                # head-of-line-block the loads on SP).
                for bi, (s, c0, ln) in enumerate(blocks):
                    ob = opool.tile([P, ln], f32, tag=f"o_{bi}", name=f"o_{u}_{bi}")
                    nc.vector.tensor_tensor(
                        out=ob[:],
                        in0=pss[bi][:],
                        in1=sc_sb[:, c0 : c0 + ln],
                        op=mybir.AluOpType.mult,
                    )
                    nc.scalar.dma_start(y[u * P : (u + 1) * P, c0 : c0 + ln], ob[:])
    nc.compile()
    return nc


def _get_nc(cap0: int, cap1: int):
    key = (cap0, cap1)
    if key not in _NC_CACHE:
        _NC_CACHE[key] = _build(cap0, cap1)
    return _NC_CACHE[key]


def _route(weights: np.ndarray):
    """Host router: top-2 charts + normalized gates (matches jax.lax.top_k
    tie-breaking: lower index first)."""
    b = weights.shape[0]
    ar = np.arange(b)
    i1 = np.argmax(weights, axis=1)
    v1 = weights[ar, i1]
    w2 = weights.copy()
    w2[ar, i1] = -np.inf
    i2 = np.argmax(w2, axis=1)
    v2 = w2[ar, i2]
    s = np.clip(v1 + v2, 1e-8, None)
    return i1, i2, v1 / s, v2 / s


def kernel(q, weights, W_stack):
    import ml_dtypes

    q = np.ascontiguousarray(np.asarray(q, dtype=np.float32))
    weights = np.ascontiguousarray(np.asarray(weights, dtype=np.float32))
    W_stack = np.asarray(W_stack, dtype=np.float32)

    B = q.shape[0]
    i1, i2, g1, g2 = _route(weights)

    # flatten the (token, chart) pairs and group by chart
    flat_chart = np.concatenate([i1, i2])
    flat_tok = np.concatenate([np.arange(B), np.arange(B)])
    flat_gate = np.concatenate([g1, g2]).astype(np.float32)
    order = np.argsort(flat_chart, kind="stable")
    counts = np.bincount(flat_chart, minlength=NUM_CHARTS)
    starts = np.zeros(NUM_CHARTS + 1, dtype=np.int64)
    starts[1:] = np.cumsum(counts)

    # balanced pairing: largest chart with smallest on the same core
    by_size = np.argsort(-counts, kind="stable")
    slot_chart = np.empty((N_CORES, 2), dtype=np.int64)
    for core in range(N_CORES):
        slot_chart[core, 0] = by_size[core]
        slot_chart[core, 1] = by_size[NUM_CHARTS - 1 - core]

    cap0 = int(counts[slot_chart[:, 0]].max())
    cap1 = int(counts[slot_chart[:, 1]].max())
    CT = cap0 + cap1

    # residual weights (W + I)^T per chart, bf16
    eye = np.eye(D, dtype=np.float32)
    Rt = (W_stack.transpose(0, 2, 1) + eye[None]).astype(ml_dtypes.bfloat16)
    q_bf = q.astype(ml_dtypes.bfloat16)

    in_maps = []
    pos = np.empty(2 * B, dtype=np.int64)  # pair j -> global Y^T column
    for core in range(N_CORES):
        xp = np.zeros((CT, D), dtype=ml_dtypes.bfloat16)  # packed tokens
        scv_c = np.zeros(CT, dtype=np.float32)
        wtm = np.empty((2, 8, P, D), dtype=ml_dtypes.bfloat16)
        for slot in range(2):
            c = int(slot_chart[core, slot])
            wtm[slot] = Rt[c].reshape(8, P, D)
            sel = order[starts[c] : starts[c + 1]]
            n = len(sel)
            col0 = slot * cap0
            xp[col0 : col0 + n] = q_bf[flat_tok[sel]]
            scv_c[col0 : col0 + n] = flat_gate[sel]
            pos[sel] = core * CT + col0 + np.arange(n)
        xtm = np.ascontiguousarray(xp.T).reshape(8, P, CT)
        in_maps.append({"xt": xtm, "wt": wtm, "scv": scv_c[None, :]})

    from concourse.bass_utils import run_bass_kernel_spmd

    nc = _get_nc(cap0, cap1)
    res = run_bass_kernel_spmd(nc, in_maps, core_ids=list(range(N_CORES)))
    # y is Y^T [1024, CT] per core; stack as [8*CT, 1024] token-major
    Yg = np.concatenate(
        [np.ascontiguousarray(np.asarray(res.results[i]["y"]).T) for i in range(N_CORES)],
        axis=0,
    )

    # exact combine: out = g1*Y1 + g2*Y2 - (g1+g2)*q   (Y are residual outs)
    gs = (g1 + g2).astype(np.float32)
    out = Yg[pos[:B]] + Yg[pos[B:]] - gs[:, None] * q
    return np.ascontiguousarray(out, dtype=np.float32)


# revision 28
# speedup vs baseline: 1.3016x; 1.0095x over previous
"""Top-k gated mixture of linear maps (MoE routing) on 8 TRN2 NeuronCores.

Expert-parallel sharding: the 16 charts are assigned 2-per-core (balanced
pairing: largest chart with smallest). Routing (top-2 chart selection +
gate normalization, an 8192x16 argmax — 0.003% of the FLOPs) runs on host
as part of the dispatch/sharding step; each core receives the tokens
routed to its two charts as transposed column blocks plus per-token gate
scales, with its two charts' residual weights resident in SBUF.

Residual decomposition: with gates g1+g2 = s, algebraically
    out = g1*W_c1 q + g2*W_c2 q = g1*(W_c1+I) q + g2*(W_c2+I) q - s*q.
The device computes the (W+I) matmuls in bf16 — since W ~= -I + 0.01*N,
the residual W+I is ~100x smaller than q's projection, so bf16 rounding
error scales with the residual, not the output (measured ~6e-4 scale-
relative absmax vs fp32 reference). Accumulation is fp32 in PSUM, gate
scaling on-device in fp32, and the exact -s*q correction is applied in
fp32 during the host unshard/combine step.

Matmul orientation: residual W^T chunks are the stationary operand and
token columns are the moving operand (out = Y^T tiles [u, tokens]), so
token capacities need no 128-alignment — per-slot capacity is exactly
the max chart load and the only padded work is capacity skew.

self-contained: only imports concourse (globally installed) + numpy.
"""

import numpy as np

P = 128
D = 1024  # DIM_Q == DIM_U
NUM_CHARTS = 16
TOP_K = 2
N_CORES = 8
MAX_MM = 512  # moving-operand / PSUM-bank limit per matmul

_NC_CACHE: dict = {}


def _blocks(cap0: int, cap1: int):
    """Static token-block list: (slot, col0, length) covering [0, cap0+cap1),
    slot-contiguous, each block <= MAX_MM, evenly split per slot."""
    out = []
    col = 0
    for s, cap in ((0, cap0), (1, cap1)):
        nb = -(-cap // MAX_MM)
        base, extra = divmod(cap, nb)
        for b in range(nb):
            ln = base + (1 if b < extra else 0)
            out.append((s, col, ln))
            col += ln
    return out


def _build(cap0: int, cap1: int):
    """Build + compile the SPMD single-core program (same on all 8 cores).

    Inputs per core (CT = cap0 + cap1):
      xt  [8, 128, CT] bf16 : X^T q-chunked: xt[qc, p, col] =
                              q[token(col)][qc*128+p]. Slot 0 tokens in
                              columns [0, cap0), slot 1 after (zero-pad).
      wt  [2, 8, 128, 1024] bf16 : (W+I)^T of the 2 charts, q-chunked the
                              same way: wt[s, qc, p, u].
      scv [1, CT] f32 : gate scale of token(col); broadcast across
                              partitions on device via k=1 matmuls.
    Output:
      y [1024, CT] f32 : Y^T — gate-scaled residual chart outputs.
    """
    import concourse.mybir as mybir
    from concourse import bacc
    from concourse.tile import TileContext

    CT = cap0 + cap1
    f32 = mybir.dt.float32
    bf16 = mybir.dt.bfloat16
    blocks = _blocks(cap0, cap1)

    nc = bacc.Bacc("TRN2", target_bir_lowering=False, debug=False)
    xt = nc.dram_tensor("xt", [8, P, CT], bf16, kind="ExternalInput")
    wt = nc.dram_tensor("wt", [2, 8, P, D], bf16, kind="ExternalInput")
    scv = nc.dram_tensor("scv", [1, CT], f32, kind="ExternalInput")
    y = nc.dram_tensor("y", [D, CT], f32, kind="ExternalOutput")

    with TileContext(nc) as tc:
        with (
            tc.tile_pool(name="wpool", bufs=1) as wpool,
            tc.tile_pool(name="xpool", bufs=1) as xpool,
            tc.tile_pool(name="spool", bufs=1) as spool,
            tc.tile_pool(name="opool", bufs=3) as opool,
            tc.tile_pool(name="psum", bufs=8, space="PSUM") as psum,
        ):
            # gate-scale broadcast [1, CT] -> [128, CT] via k=1 matmuls with
            # a ones column; cheap, and warms the PE during the preload.
            sc_row = spool.tile([1, CT], f32, name="sc_row")
            nc.scalar.dma_start(sc_row[:], scv[:])
            ones = spool.tile([1, P], f32, name="ones")
            nc.vector.memset(ones[:], 1.0)
            sc_sb = spool.tile([P, CT], f32, name="sc_sb")
            for bi, (_s, c0, ln) in enumerate(blocks):
                bps = psum.tile([P, ln], f32, tag="ps", name=f"bps_{bi}")
                nc.tensor.matmul(
                    bps[:],
                    lhsT=ones[:],
                    rhs=sc_row[:, c0 : c0 + ln],
                    start=True,
                    stop=True,
                )
                nc.vector.tensor_copy(sc_sb[:, c0 : c0 + ln], bps[:])

            # X^T resident: 8 q-chunk tiles [128, CT]; residual W^T resident:
            # 8 per-q-chunk tiles per slot (fine-grained dependencies). Issue
            # order: (w0,x,w1) per q-chunk so accumulation over qc can chase
            # the arrival front.
            x_sb = [None] * 8
            w_sb = {0: [None] * 8, 1: [None] * 8}

            def _load_x(qc):
                t = xpool.tile([P, CT], bf16, tag=f"x_{qc}", name=f"x_{qc}")
                nc.sync.dma_start(t[:], xt[qc])
                x_sb[qc] = t

            def _load_w(s, qc):
                t = wpool.tile([P, D], bf16, tag=f"w_{s}_{qc}", name=f"w_{s}_{qc}")
                nc.sync.dma_start(t[:], wt[s, qc])
                w_sb[s][qc] = t

            for qc in range(8):
                _load_w(0, qc)
                _load_x(qc)
                _load_w(1, qc)

            for u in range(8):
                pss = [
                    psum.tile([P, ln], f32, tag="ps", name=f"ps_{u}_{bi}")
                    for bi, (_s, _c, ln) in enumerate(blocks)
                ]
                # qc outer, blocks inner: the stationary operand changes only
                # twice per qc (once per slot), and accumulation can chase the
                # X-chunk arrival front during the preload. The final column
                # runs block-outer instead so the tail evictions pipeline with
                # the remaining matmuls.
                if u < 7:
                    mm_order = [(qc, bi) for qc in range(8) for bi in range(len(blocks))]
                else:
                    mm_order = [(qc, bi) for bi in range(len(blocks)) for qc in range(8)]
                for qc, bi in mm_order:
                    s, c0, ln = blocks[bi]
                    nc.tensor.matmul(
                        pss[bi][:],
                        lhsT=w_sb[s][qc][:, u * P : (u + 1) * P],
                        rhs=x_sb[qc][:, c0 : c0 + ln],
                        start=(qc == 0),
                        stop=(qc == 7),
                    )
                # per-block eviction into a shared row tile; one Y^T DMA per
                # slot half (ACT queue, so the waits don't head-of-line-block
                # the loads on SP).
                ot = opool.tile([P, CT], f32, tag="o", name=f"o_{u}")
                prev_s = 0
                for bi, (s, c0, ln) in enumerate(blocks):
                    if s != prev_s:
                        nc.scalar.dma_start(
                            y[u * P : (u + 1) * P, 0:cap0], ot[:, 0:cap0]
                        )
                        prev_s = s
                    nc.vector.tensor_tensor(
                        out=ot[:, c0 : c0 + ln],
                        in0=pss[bi][:],
                        in1=sc_sb[:, c0 : c0 + ln],
                        op=mybir.AluOpType.mult,
                    )
                nc.scalar.dma_start(y[u * P : (u + 1) * P, cap0:CT], ot[:, cap0:CT])
    nc.compile()
    return nc


def _get_nc(cap0: int, cap1: int):
    key = (cap0, cap1)
    if key not in _NC_CACHE:
        _NC_CACHE[key] = _build(cap0, cap1)
    return _NC_CACHE[key]


def _route(weights: np.ndarray):
    """Host router: top-2 charts + normalized gates (matches jax.lax.top_k
    tie-breaking: lower index first)."""
    b = weights.shape[0]
    ar = np.arange(b)
    i1 = np.argmax(weights, axis=1)
    v1 = weights[ar, i1]
    w2 = weights.copy()
    w2[ar, i1] = -np.inf
    i2 = np.argmax(w2, axis=1)
    v2 = w2[ar, i2]
    s = np.clip(v1 + v2, 1e-8, None)
    return i1, i2, v1 / s, v2 / s


def kernel(q, weights, W_stack):
    import ml_dtypes

    q = np.ascontiguousarray(np.asarray(q, dtype=np.float32))
    weights = np.ascontiguousarray(np.asarray(weights, dtype=np.float32))
    W_stack = np.asarray(W_stack, dtype=np.float32)

    B = q.shape[0]
    i1, i2, g1, g2 = _route(weights)

    # flatten the (token, chart) pairs and group by chart
    flat_chart = np.concatenate([i1, i2])
    flat_tok = np.concatenate([np.arange(B), np.arange(B)])
    flat_gate = np.concatenate([g1, g2]).astype(np.float32)
    order = np.argsort(flat_chart, kind="stable")
    counts = np.bincount(flat_chart, minlength=NUM_CHARTS)
    starts = np.zeros(NUM_CHARTS + 1, dtype=np.int64)
    starts[1:] = np.cumsum(counts)

    # balanced pairing: largest chart with smallest on the same core
    by_size = np.argsort(-counts, kind="stable")
    slot_chart = np.empty((N_CORES, 2), dtype=np.int64)
    for core in range(N_CORES):
        slot_chart[core, 0] = by_size[core]
        slot_chart[core, 1] = by_size[NUM_CHARTS - 1 - core]

    cap0 = int(counts[slot_chart[:, 0]].max())
    cap1 = int(counts[slot_chart[:, 1]].max())
    CT = cap0 + cap1

    # residual weights (W + I)^T per chart, bf16
    eye = np.eye(D, dtype=np.float32)
    Rt = (W_stack.transpose(0, 2, 1) + eye[None]).astype(ml_dtypes.bfloat16)
    q_bf = q.astype(ml_dtypes.bfloat16)

    in_maps = []
    pos = np.empty(2 * B, dtype=np.int64)  # pair j -> global Y^T column
    for core in range(N_CORES):
        xp = np.zeros((CT, D), dtype=ml_dtypes.bfloat16)  # packed tokens
        scv_c = np.zeros(CT, dtype=np.float32)
        wtm = np.empty((2, 8, P, D), dtype=ml_dtypes.bfloat16)
        for slot in range(2):
            c = int(slot_chart[core, slot])
            wtm[slot] = Rt[c].reshape(8, P, D)
            sel = order[starts[c] : starts[c + 1]]
            n = len(sel)
            col0 = slot * cap0
            xp[col0 : col0 + n] = q_bf[flat_tok[sel]]
            scv_c[col0 : col0 + n] = flat_gate[sel]
            pos[sel] = core * CT + col0 + np.arange(n)
        xtm = np.ascontiguousarray(xp.T).reshape(8, P, CT)
        in_maps.append({"xt": xtm, "wt": wtm, "scv": scv_c[None, :]})

    from concourse.bass_utils import run_bass_kernel_spmd

    nc = _get_nc(cap0, cap1)
    res = run_bass_kernel_spmd(nc, in_maps, core_ids=list(range(N_CORES)))
    # y is Y^T [1024, CT] per core; stack as [8*CT, 1024] token-major
    Yg = np.concatenate(
        [np.ascontiguousarray(np.asarray(res.results[i]["y"]).T) for i in range(N_CORES)],
        axis=0,
    )

    # exact combine: out = g1*Y1 + g2*Y2 - (g1+g2)*q   (Y are residual outs)
    gs = (g1 + g2).astype(np.float32)
    out = Yg[pos[:B]] + Yg[pos[B:]] - gs[:, None] * q
    return np.ascontiguousarray(out, dtype=np.float32)


# revision 29
# speedup vs baseline: 1.3157x; 1.0108x over previous
"""Top-k gated mixture of linear maps (MoE routing) on 8 TRN2 NeuronCores.

Expert-parallel sharding: the 16 charts are assigned 2-per-core (balanced
pairing: largest chart with smallest). Routing (top-2 chart selection +
gate normalization, an 8192x16 argmax — 0.003% of the FLOPs) runs on host
as part of the dispatch/sharding step; each core receives the tokens
routed to its two charts as transposed column blocks plus per-token gate
scales, with its two charts' residual weights resident in SBUF.

Residual decomposition: with gates g1+g2 = s, algebraically
    out = g1*W_c1 q + g2*W_c2 q = g1*(W_c1+I) q + g2*(W_c2+I) q - s*q.
The device computes the (W+I) matmuls in bf16 — since W ~= -I + 0.01*N,
the residual W+I is ~100x smaller than q's projection, so bf16 rounding
error scales with the residual, not the output (measured ~6e-4 scale-
relative absmax vs fp32 reference). Accumulation is fp32 in PSUM, gate
scaling on-device in fp32, and the exact -s*q correction is applied in
fp32 during the host unshard/combine step.

Matmul orientation: residual W^T chunks are the stationary operand and
token columns are the moving operand (out = Y^T tiles [u, tokens]), so
token capacities need no 128-alignment — per-slot capacity is exactly
the max chart load and the only padded work is capacity skew.

self-contained: only imports concourse (globally installed) + numpy.
"""

import numpy as np

P = 128
D = 1024  # DIM_Q == DIM_U
NUM_CHARTS = 16
TOP_K = 2
N_CORES = 8
MAX_MM = 512  # moving-operand / PSUM-bank limit per matmul

_NC_CACHE: dict = {}


def _blocks(cap0: int, cap1: int):
    """Static token-block list: (slot, col0, length) covering [0, cap0+cap1),
    slot-contiguous, each block <= MAX_MM, evenly split per slot."""
    out = []
    col = 0
    for s, cap in ((0, cap0), (1, cap1)):
        nb = -(-cap // MAX_MM)
        base, extra = divmod(cap, nb)
        for b in range(nb):
            ln = base + (1 if b < extra else 0)
            out.append((s, col, ln))
            col += ln
    return out


def _build(cap0: int, cap1: int):
    """Build + compile the SPMD single-core program (same on all 8 cores).

    Inputs per core (CT = cap0 + cap1):
      xt  [8, 128, CT] bf16 : X^T q-chunked: xt[qc, p, col] =
                              q[token(col)][qc*128+p]. Slot 0 tokens in
                              columns [0, cap0), slot 1 after (zero-pad).
      wt  [2, 8, 128, 1024] bf16 : (W+I)^T of the 2 charts, q-chunked the
                              same way: wt[s, qc, p, u].
      scv [1, CT] f32 : gate scale of token(col); broadcast across
                              partitions on device via k=1 matmuls.
    Output:
      y [1024, CT] f32 : Y^T — gate-scaled residual chart outputs.
    """
    import concourse.mybir as mybir
    from concourse import bacc
    from concourse.tile import TileContext

    CT = cap0 + cap1
    f32 = mybir.dt.float32
    bf16 = mybir.dt.bfloat16
    blocks = _blocks(cap0, cap1)

    nc = bacc.Bacc("TRN2", target_bir_lowering=False, debug=False)
    xt = nc.dram_tensor("xt", [8, P, CT], bf16, kind="ExternalInput")
    wt = nc.dram_tensor("wt", [2, 8, P, D], bf16, kind="ExternalInput")
    scv = nc.dram_tensor("scv", [1, CT], f32, kind="ExternalInput")
    y = nc.dram_tensor("y", [D, CT], f32, kind="ExternalOutput")

    with TileContext(nc) as tc:
        with (
            tc.tile_pool(name="wpool", bufs=1) as wpool,
            tc.tile_pool(name="xpool", bufs=1) as xpool,
            tc.tile_pool(name="spool", bufs=1) as spool,
            tc.tile_pool(name="opool", bufs=3) as opool,
            tc.tile_pool(name="psum", bufs=8, space="PSUM") as psum,
        ):
            # gate-scale broadcast [1, CT] -> [128, CT] via k=1 matmuls with
            # a ones column; cheap, and warms the PE during the preload.
            sc_row = spool.tile([1, CT], f32, name="sc_row")
            nc.scalar.dma_start(sc_row[:], scv[:])
            ones = spool.tile([1, P], f32, name="ones")
            nc.vector.memset(ones[:], 1.0)
            sc_sb = spool.tile([P, CT], f32, name="sc_sb")
            for bi, (_s, c0, ln) in enumerate(blocks):
                bps = psum.tile([P, ln], f32, tag="ps", name=f"bps_{bi}")
                nc.tensor.matmul(
                    bps[:],
                    lhsT=ones[:],
                    rhs=sc_row[:, c0 : c0 + ln],
                    start=True,
                    stop=True,
                )
                nc.vector.tensor_copy(sc_sb[:, c0 : c0 + ln], bps[:])

            # X^T resident: 8 q-chunk tiles [128, CT]; residual W^T resident:
            # 8 per-q-chunk tiles per slot (fine-grained dependencies). Issue
            # order: (w0,x,w1) per q-chunk so accumulation over qc can chase
            # the arrival front.
            x_sb = [None] * 8
            w_sb = {0: [None] * 8, 1: [None] * 8}

            def _load_x(qc):
                t = xpool.tile([P, CT], bf16, tag=f"x_{qc}", name=f"x_{qc}")
                nc.sync.dma_start(t[:], xt[qc])
                x_sb[qc] = t

            def _load_w(s, qc):
                t = wpool.tile([P, D], bf16, tag=f"w_{s}_{qc}", name=f"w_{s}_{qc}")
                nc.sync.dma_start(t[:], wt[s, qc])
                w_sb[s][qc] = t

            for qc in range(8):
                _load_w(0, qc)
                _load_x(qc)
                _load_w(1, qc)

            for u in range(8):
                pss = [
                    psum.tile([P, ln], f32, tag="ps", name=f"ps_{u}_{bi}")
                    for bi, (_s, _c, ln) in enumerate(blocks)
                ]
                # qc outer, blocks inner: the stationary operand changes only
                # twice per qc (once per slot), and accumulation can chase the
                # X-chunk arrival front during the preload. The final column
                # runs block-outer instead so the tail evictions pipeline with
                # the remaining matmuls.
                if u < 7:
                    mm_order = [(qc, bi) for qc in range(8) for bi in range(len(blocks))]
                else:
                    mm_order = [(qc, bi) for bi in range(len(blocks)) for qc in range(8)]
                for qc, bi in mm_order:
                    s, c0, ln = blocks[bi]
                    nc.tensor.matmul(
                        pss[bi][:],
                        lhsT=w_sb[s][qc][:, u * P : (u + 1) * P],
                        rhs=x_sb[qc][:, c0 : c0 + ln],
                        start=(qc == 0),
                        stop=(qc == 7),
                    )
                # per-block eviction into a shared row tile; one Y^T DMA per
                # slot half (ACT queue, so the waits don't head-of-line-block
                # the loads on SP). The final column stores per block so the
                # tail DMA pipelines with the last evictions.
                ot = opool.tile([P, CT], f32, tag="o", name=f"o_{u}")
                prev_s = 0
                for bi, (s, c0, ln) in enumerate(blocks):
                    if u < 7 and s != prev_s:
                        nc.scalar.dma_start(
                            y[u * P : (u + 1) * P, 0:cap0], ot[:, 0:cap0]
                        )
                        prev_s = s
                    nc.vector.tensor_tensor(
                        out=ot[:, c0 : c0 + ln],
                        in0=pss[bi][:],
                        in1=sc_sb[:, c0 : c0 + ln],
                        op=mybir.AluOpType.mult,
                    )
                    if u == 7:
                        nc.scalar.dma_start(
                            y[u * P : (u + 1) * P, c0 : c0 + ln], ot[:, c0 : c0 + ln]
                        )
                if u < 7:
                    nc.scalar.dma_start(
                        y[u * P : (u + 1) * P, cap0:CT], ot[:, cap0:CT]
                    )
    nc.compile()
    return nc


def _get_nc(cap0: int, cap1: int):
    key = (cap0, cap1)
    if key not in _NC_CACHE:
        _NC_CACHE[key] = _build(cap0, cap1)
    return _NC_CACHE[key]


def _route(weights: np.ndarray):
    """Host router: top-2 charts + normalized gates (matches jax.lax.top_k
    tie-breaking: lower index first)."""
    b = weights.shape[0]
    ar = np.arange(b)
    i1 = np.argmax(weights, axis=1)
    v1 = weights[ar, i1]
    w2 = weights.copy()
    w2[ar, i1] = -np.inf
    i2 = np.argmax(w2, axis=1)
    v2 = w2[ar, i2]
    s = np.clip(v1 + v2, 1e-8, None)
    return i1, i2, v1 / s, v2 / s


def kernel(q, weights, W_stack):
    import ml_dtypes

    q = np.ascontiguousarray(np.asarray(q, dtype=np.float32))
    weights = np.ascontiguousarray(np.asarray(weights, dtype=np.float32))
    W_stack = np.asarray(W_stack, dtype=np.float32)

    B = q.shape[0]
    i1, i2, g1, g2 = _route(weights)

    # flatten the (token, chart) pairs and group by chart
    flat_chart = np.concatenate([i1, i2])
    flat_tok = np.concatenate([np.arange(B), np.arange(B)])
    flat_gate = np.concatenate([g1, g2]).astype(np.float32)
    order = np.argsort(flat_chart, kind="stable")
    counts = np.bincount(flat_chart, minlength=NUM_CHARTS)
    starts = np.zeros(NUM_CHARTS + 1, dtype=np.int64)
    starts[1:] = np.cumsum(counts)

    # balanced pairing: largest chart with smallest on the same core
    by_size = np.argsort(-counts, kind="stable")
    slot_chart = np.empty((N_CORES, 2), dtype=np.int64)
    for core in range(N_CORES):
        slot_chart[core, 0] = by_size[core]
        slot_chart[core, 1] = by_size[NUM_CHARTS - 1 - core]

    cap0 = int(counts[slot_chart[:, 0]].max())
    cap1 = int(counts[slot_chart[:, 1]].max())
    CT = cap0 + cap1

    # residual weights (W + I)^T per chart, bf16
    eye = np.eye(D, dtype=np.float32)
    Rt = (W_stack.transpose(0, 2, 1) + eye[None]).astype(ml_dtypes.bfloat16)
    q_bf = q.astype(ml_dtypes.bfloat16)

    in_maps = []
    pos = np.empty(2 * B, dtype=np.int64)  # pair j -> global Y^T column
    for core in range(N_CORES):
        xp = np.zeros((CT, D), dtype=ml_dtypes.bfloat16)  # packed tokens
        scv_c = np.zeros(CT, dtype=np.float32)
        wtm = np.empty((2, 8, P, D), dtype=ml_dtypes.bfloat16)
        for slot in range(2):
            c = int(slot_chart[core, slot])
            wtm[slot] = Rt[c].reshape(8, P, D)
            sel = order[starts[c] : starts[c + 1]]
            n = len(sel)
            col0 = slot * cap0
            xp[col0 : col0 + n] = q_bf[flat_tok[sel]]
            scv_c[col0 : col0 + n] = flat_gate[sel]
            pos[sel] = core * CT + col0 + np.arange(n)
        xtm = np.ascontiguousarray(xp.T).reshape(8, P, CT)
        in_maps.append({"xt": xtm, "wt": wtm, "scv": scv_c[None, :]})

    from concourse.bass_utils import run_bass_kernel_spmd

    nc = _get_nc(cap0, cap1)
    res = run_bass_kernel_spmd(nc, in_maps, core_ids=list(range(N_CORES)))
    # y is Y^T [1024, CT] per core; stack as [8*CT, 1024] token-major
    Yg = np.concatenate(
        [np.ascontiguousarray(np.asarray(res.results[i]["y"]).T) for i in range(N_CORES)],
        axis=0,
    )

    # exact combine: out = g1*Y1 + g2*Y2 - (g1+g2)*q   (Y are residual outs)
    gs = (g1 + g2).astype(np.float32)
    out = Yg[pos[:B]] + Yg[pos[B:]] - gs[:, None] * q
    return np.ascontiguousarray(out, dtype=np.float32)
